# revision 2
# baseline (speedup 1.0000x reference)
"""Trainium2 Bass kernel for the BMoIE (dense mixture-of-experts) network.

Network (per sample):
    alpha = softmax(x @ gate_w + gate_b)                       # [B, 8]
    h = relu(sum_k alpha_k * (h @ w_l[k] + b_l[k]))            # 3 hidden blocks
    out = sum_k alpha_k * (h @ wo[k] + bo[k])                  # output block

Strategy: data-parallel over 8 NeuronCores (2048 rows each, SPMD, no
collectives). On each core, activations are kept feature-major ("hT",
[128 feat-partitions x batch]) so they can be the stationary matmul
operand; expert weights stream in natural layout as the moving operand,
producing batch-major per-expert outputs in PSUM. The alpha-weighted sum
over the 8 experts runs on VectorE as scalar_tensor_tensor with the
per-partition (per-sample) alpha scalar. Matmuls use float32r (FP22
multiply, fp32 accumulate) which runs at full PE speed. Block outputs are
re-transposed to feature-major with PE transposes for the next layer.
"""

import sys

sys.path.insert(0, "/opt/trn_rl_repo")

import numpy as np

import concourse.bass as bass
import concourse.mybir as mybir
import concourse.tile as tile
from concourse import bacc
from concourse.bass_utils import run_bass_kernel_spmd
from concourse.masks import make_identity

P = 128           # partitions
D = 512           # model dim (= hidden dim)
K = 8             # experts
NCORES = 8
B = 16384
R = B // NCORES   # rows per core
NT = R // P       # 16 batch tiles per core
NCH = D // P      # 4 feature chunks
F32 = mybir.dt.float32
FR = mybir.dt.float32r
AF = mybir.ActivationFunctionType
ALU = mybir.AluOpType

W_NAMES = ("w0", "w1", "w2", "wo")
B_NAMES = ("b0", "b1", "b2", "bo")


def _build(has_gate_b, has_bias, w_bufs=44, ht_bufs=22, z_bufs=6):
    """Trace + compile the per-core kernel. has_bias is a 4-tuple of bools."""
    from contextlib import ExitStack

    nc = bacc.Bacc("TRN2", target_bir_lowering=False, num_devices=NCORES)
    x = nc.dram_tensor("x", [R, D], F32, kind="ExternalInput")
    gate_w = nc.dram_tensor("gate_w", [D, K], FR, kind="ExternalInput")
    gate_b = nc.dram_tensor("gate_b", [K], F32, kind="ExternalInput")
    ws = [nc.dram_tensor(n, [K, D, D], FR, kind="ExternalInput") for n in W_NAMES]
    bs = [nc.dram_tensor(n, [K, D], FR, kind="ExternalInput") for n in B_NAMES]
    out = nc.dram_tensor("out", [R, D], F32, kind="ExternalOutput")

    any_bias = any(has_bias)

    with tile.TileContext(nc) as tc, ExitStack() as ctx:
        cst = ctx.enter_context(tc.tile_pool(name="cst", bufs=1))
        wpool = ctx.enter_context(tc.tile_pool(name="wpool", bufs=w_bufs))
        htp = ctx.enter_context(tc.tile_pool(name="htp", bufs=ht_bufs))
        accp = ctx.enter_context(tc.tile_pool(name="accp", bufs=3))
        xbp = ctx.enter_context(tc.tile_pool(name="xbp", bufs=3))
        smp = ctx.enter_context(tc.tile_pool(name="smp", bufs=4))
        zp = ctx.enter_context(tc.tile_pool(name="zp", bufs=z_bufs, space="PSUM"))
        trp = ctx.enter_context(tc.tile_pool(name="trp", bufs=8 - z_bufs, space="PSUM"))

        ident = cst.tile([P, P], F32, tag="ident")
        make_identity(nc, ident[:])

        # gate_w [512, 8] -> [128, 4*8] (chunk c at cols c*8..)
        gw = cst.tile([P, NCH * K], FR, tag="gw")
        for c in range(NCH):
            nc.sync.dma_start(gw[:, c * K:(c + 1) * K], gate_w[c * P:(c + 1) * P, :])

        gb_bc = None
        if has_gate_b:
            ones_row = cst.tile([1, P], F32, tag="ones_row")
            nc.vector.memset(ones_row[:], 1.0)
            gb_row = cst.tile([1, K], F32, tag="gb_row")
            nc.sync.dma_start(gb_row[:], gate_b[None, :])
            gb_ps = trp.tile([P, D], F32, tag="tr")
            nc.tensor.matmul(gb_ps[:, :K], ones_row[:], gb_row[:])
            gb_bc = cst.tile([P, K], F32, tag="gb_bc")
            nc.scalar.activation(gb_bc[:], gb_ps[:, :K], AF.Copy)

        bl_sb = [None] * 4
        if any_bias:
            for li in range(4):
                if has_bias[li]:
                    blt = cst.tile([K, D], FR, tag=f"bl{li}")
                    nc.sync.dma_start(blt[:], bs[li][:, :])
                    bl_sb[li] = blt
            alphaT = cst.tile([K, R], FR, tag="alphaT")

        alpha = cst.tile([P, NT * K], F32, tag="alpha")

        # ---- prologue: load x tiles, transpose to feature-major, gate ----
        hT = {}
        for t in range(NT):
            xb = xbp.tile([P, D], F32, tag="xb")
            nc.sync.dma_start(xb[:], x[t * P:(t + 1) * P, :])
            trt = trp.tile([P, NCH, P], F32, tag="tr")
            for c in range(NCH):
                nc.tensor.transpose(trt[:, c, :], xb[:, c * P:(c + 1) * P], ident[:])
            ht = htp.tile([P, NCH, P], FR, tag="ht")
            nc.scalar.activation(ht[:], trt[:], AF.Copy)
            hT[(0, t)] = ht

            # gate for this tile (batch-major logits)
            lg = zp.tile([P, D], F32, tag="z")
            for c in range(NCH):
                nc.tensor.matmul(
                    lg[:, :K],
                    ht[:, c, :],
                    gw[:, c * K:(c + 1) * K],
                    start=(c == 0),
                    stop=(c == NCH - 1),
                )
            ex = smp.tile([P, K], F32, tag="ex")
            if has_gate_b:
                nc.vector.scalar_tensor_tensor(
                    ex[:], lg[:, :K], 1.0, gb_bc[:], op0=ALU.mult, op1=ALU.add
                )
                nc.scalar.activation(ex[:], ex[:], AF.Exp)
            else:
                nc.scalar.activation(ex[:], lg[:, :K], AF.Exp)
            ssum = smp.tile([P, 1], F32, tag="ssum")
            nc.vector.reduce_sum(ssum[:], ex[:], axis=mybir.AxisListType.X)
            rec = smp.tile([P, 1], F32, tag="rec")
            nc.vector.reciprocal(rec[:], ssum[:])
            nc.vector.tensor_scalar_mul(alpha[:, t * K:(t + 1) * K], ex[:], rec[:])

            if any_bias:
                # alphaT[:, t*128:(t+1)*128] = alpha_tile.T (8 x 128)
                at_ps = trp.tile([P, NCH, P], F32, tag="tr")
                nc.tensor.transpose(
                    at_ps[:K, 0, :], alpha[:, t * K:(t + 1) * K], ident[:]
                )
                nc.scalar.activation(
                    alphaT[:, t * P:(t + 1) * P], at_ps[:K, 0, :], AF.Copy
                )

        # ---- 4 MoIE blocks ----
        for li in range(4):
            # stream this layer's weights (reused across all 16 batch tiles)
            wt = {}
            for k in range(K):
                for c in range(NCH):
                    w_t = wpool.tile([P, D], FR, tag="w", name=f"w_{li}_{k}_{c}")
                    nc.sync.dma_start(w_t[:], ws[li][k, c * P:(c + 1) * P, :])
                    wt[(k, c)] = w_t

            for t in range(NT):
                bias_sb = None
                if has_bias[li]:
                    b_ps = trp.tile([P, D], F32, tag="tr")
                    nc.tensor.matmul(
                        b_ps[:],
                        alphaT[:, t * P:(t + 1) * P],
                        bl_sb[li][:],
                    )
                    bias_sb = smp.tile([P, D], F32, tag="bias_sb")
                    nc.scalar.activation(bias_sb[:], b_ps[:], AF.Copy)

                ht_in = hT[(li, t)]
                zs = []
                for k in range(K):
                    z = zp.tile([P, D], F32, tag="z", name=f"z_{li}_{t}_{k}")
                    for c in range(NCH):
                        nc.tensor.matmul(
                            z[:],
                            ht_in[:, c, :],
                            wt[(k, c)][:],
                            start=(c == 0),
                            stop=(c == NCH - 1),
                        )
                    zs.append(z)

                # weighted sum over experts on VectorE (per-partition alpha)
                acc = accp.tile([P, D], F32, tag="acc")
                for k in range(K):
                    a_ap = alpha[:, t * K + k:t * K + k + 1]
                    if k == 0:
                        if bias_sb is not None:
                            nc.vector.scalar_tensor_tensor(
                                acc[:], zs[0][:], a_ap, bias_sb[:],
                                op0=ALU.mult, op1=ALU.add,
                            )
                        else:
                            nc.vector.tensor_scalar_mul(acc[:], zs[0][:], a_ap)
                    else:
                        nc.vector.scalar_tensor_tensor(
                            acc[:], zs[k][:], a_ap, acc[:],
                            op0=ALU.mult, op1=ALU.add,
                        )

                if li < 3:
                    nc.scalar.activation(acc[:], acc[:], AF.Relu)
                    trt = trp.tile([P, NCH, P], F32, tag="tr")
                    for c in range(NCH):
                        nc.tensor.transpose(
                            trt[:, c, :], acc[:, c * P:(c + 1) * P], ident[:]
                        )
                    ht_n = htp.tile([P, NCH, P], FR, tag="ht")
                    nc.scalar.activation(ht_n[:], trt[:], AF.Copy)
                    hT[(li + 1, t)] = ht_n
                else:
                    nc.sync.dma_start(out[t * P:(t + 1) * P, :], acc[:])

    nc.compile()
    return nc


_CACHE = {}


def _get_nc(key):
    if key not in _CACHE:
        _CACHE[key] = _build(key[0], key[1])
    return _CACHE[key]


def kernel(**inputs):
    x = np.ascontiguousarray(np.asarray(inputs["x"], dtype=np.float32))
    gate_w = np.ascontiguousarray(np.asarray(inputs["gate_w"], dtype=np.float32))
    gate_b = np.ascontiguousarray(np.asarray(inputs["gate_b"], dtype=np.float32))
    wlist = [np.ascontiguousarray(np.asarray(inputs[n], dtype=np.float32)) for n in W_NAMES]
    blist = [np.ascontiguousarray(np.asarray(inputs[n], dtype=np.float32)) for n in B_NAMES]

    has_gate_b = bool(np.any(gate_b))
    has_bias = tuple(bool(np.any(b)) for b in blist)
    nc = _get_nc((has_gate_b, has_bias))

    shared = {"gate_w": gate_w, "gate_b": gate_b}
    for n, w in zip(W_NAMES, wlist):
        shared[n] = w
    for n, b in zip(B_NAMES, blist):
        shared[n] = b

    core_ids = list(range(NCORES))
    in_maps = [dict(shared, x=x[i * R:(i + 1) * R]) for i in core_ids]
    res = run_bass_kernel_spmd(nc, in_maps, core_ids)
    return np.concatenate([res.results[i]["out"] for i in core_ids], axis=0)


if __name__ == "__main__":
    rng = np.random.default_rng(0)
    ins = {
        "x": rng.standard_normal((B, D), dtype=np.float32),
        "gate_w": rng.standard_normal((D, K), dtype=np.float32) * 0.02,
        "gate_b": np.zeros((K,), np.float32),
    }
    for n in W_NAMES:
        ins[n] = rng.standard_normal((K, D, D), dtype=np.float32) * 0.02
    for n in B_NAMES:
        ins[n] = np.zeros((K, D), np.float32)
    y = kernel(**ins)
    print("out", y.shape, y.dtype, float(np.abs(y).max()))


# revision 14
# speedup vs baseline: 5.8531x; 5.8531x over previous
"""Trainium2 Bass kernel for the BMoIE (dense mixture-of-experts) network.

Network (per sample):
    alpha = softmax(x @ gate_w + gate_b)                       # [B, 8]
    h = relu(sum_k alpha_k * (h @ w_l[k] + b_l[k]))            # 3 hidden blocks
    out = sum_k alpha_k * (h @ wo[k] + bo[k])                  # output block

Strategy: data-parallel over 8 NeuronCores (2048 rows each, SPMD, no
collectives). On each core, activations are kept feature-major ("hT",
[128 feat-partitions x batch]) so they can be the stationary matmul
operand; expert weights stream in natural layout as the moving operand,
producing batch-major per-expert outputs in PSUM. The alpha-weighted sum
over the 8 experts runs on VectorE as scalar_tensor_tensor with the
per-partition (per-sample) alpha scalar. Matmuls use float32r (FP22
multiply, fp32 accumulate) which runs at full PE speed. Block outputs are
re-transposed to feature-major with PE transposes for the next layer.
"""

import sys

sys.path.insert(0, "/opt/trn_rl_repo")

import numpy as np

import concourse.bass as bass
import concourse.mybir as mybir
import concourse.tile as tile
from concourse import bacc
from concourse.bass_utils import run_bass_kernel_spmd
from concourse.masks import make_identity

P = 128           # partitions
D = 512           # model dim (= hidden dim)
K = 8             # experts
NCORES = 8
B = 16384
R = B // NCORES   # rows per core
NT = R // P       # 16 batch tiles per core
NCH = D // P      # 4 feature chunks
F32 = mybir.dt.float32
FR = mybir.dt.float32r
AF = mybir.ActivationFunctionType
ALU = mybir.AluOpType

W_NAMES = ("w0", "w1", "w2", "wo")
B_NAMES = ("b0", "b1", "b2", "bo")


def _build(has_gate_b, has_bias, w_bufs=64, ht_bufs=20, z_bufs=6, repeat=1,
           mode="full", gp_offload=0, bf16=False):
    """Trace + compile the per-core kernel. has_bias is a 4-tuple of bools.

    repeat>1 runs the whole 4-layer stack that many times (same weights,
    full DMA traffic each time) — used only for timing measurements.
    mode: "full" | "pe_only" (no combine/evict) | "no_stt" (ACT evict
    instead of combine) | "no_tr" (skip transposes, reuse layer-0 hT) —
    ablation variants for timing only (wrong results except "full").
    gp_offload: number of experts whose scale runs on ScalarE with the
    partial sum on GpSimd (reduces VectorE op count).
    """
    from contextlib import ExitStack

    MD = mybir.dt.bfloat16 if bf16 else FR  # matmul operand dtype

    nc = bacc.Bacc("TRN2", target_bir_lowering=False, num_devices=NCORES)
    x = nc.dram_tensor("x", [R, D], F32, kind="ExternalInput")
    gate_w = nc.dram_tensor("gate_w", [D, K], MD, kind="ExternalInput")
    gate_b = nc.dram_tensor("gate_b", [K], F32, kind="ExternalInput")
    ws = [nc.dram_tensor(n, [K, D, D], MD, kind="ExternalInput") for n in W_NAMES]
    bs = [nc.dram_tensor(n, [K, D], FR, kind="ExternalInput") for n in B_NAMES]
    out = nc.dram_tensor("out", [R, D], F32, kind="ExternalOutput")

    any_bias = any(has_bias)

    if mode == "trivial":
        with tile.TileContext(nc) as tc, ExitStack() as ctx:
            pool = ctx.enter_context(tc.tile_pool(name="triv", bufs=2))
            tt = pool.tile([P, D], F32, tag="tt")
            nc.sync.dma_start(tt[:], x[0:P, :])
            nc.sync.dma_start(out[0:P, :], tt[:])
        nc.compile()
        return nc

    with tile.TileContext(nc) as tc, ExitStack() as ctx:
        cst = ctx.enter_context(tc.tile_pool(name="cst", bufs=1))
        wpool = ctx.enter_context(tc.tile_pool(name="wpool", bufs=w_bufs))
        htp = ctx.enter_context(tc.tile_pool(name="htp", bufs=ht_bufs))
        accp = ctx.enter_context(tc.tile_pool(name="accp", bufs=3))
        xbp = ctx.enter_context(tc.tile_pool(name="xbp", bufs=3))
        smp = ctx.enter_context(tc.tile_pool(name="smp", bufs=4))
        zp = ctx.enter_context(tc.tile_pool(name="zp", bufs=z_bufs, space="PSUM"))
        trp = ctx.enter_context(tc.tile_pool(name="trp", bufs=8 - z_bufs, space="PSUM"))

        ident = cst.tile([P, P], F32, tag="ident")
        make_identity(nc, ident[:])

        # gate_w [512, 8] -> [128, 4*8] (chunk c at cols c*8..)
        gw = cst.tile([P, NCH * K], MD, tag="gw")
        for c in range(NCH):
            nc.sync.dma_start(gw[:, c * K:(c + 1) * K], gate_w[c * P:(c + 1) * P, :])

        gb_bc = None
        if has_gate_b:
            ones_row = cst.tile([1, P], F32, tag="ones_row")
            nc.vector.memset(ones_row[:], 1.0)
            gb_row = cst.tile([1, K], F32, tag="gb_row")
            nc.sync.dma_start(gb_row[:], gate_b[None, :])
            gb_ps = trp.tile([P, D], F32, tag="tr")
            nc.tensor.matmul(gb_ps[:, :K], ones_row[:], gb_row[:])
            gb_bc = cst.tile([P, K], F32, tag="gb_bc")
            nc.scalar.activation(gb_bc[:], gb_ps[:, :K], AF.Copy)

        bl_sb = [None] * 4
        if any_bias:
            for li in range(4):
                if has_bias[li]:
                    blt = cst.tile([K, D], FR, tag=f"bl{li}")
                    nc.sync.dma_start(blt[:], bs[li][:, :])
                    bl_sb[li] = blt
            alphaT = cst.tile([K, R], FR, tag="alphaT")

        alpha = cst.tile([P, NT * K], F32, tag="alpha")

        # ---- prologue: load x tiles, transpose to feature-major, gate ----
        hT = {}
        for t in range(NT):
            xb = xbp.tile([P, D], F32, tag="xb")
            nc.sync.dma_start(xb[:], x[t * P:(t + 1) * P, :])
            trt = trp.tile([P, NCH, P], F32, tag="tr")
            for c in range(NCH):
                nc.tensor.transpose(trt[:, c, :], xb[:, c * P:(c + 1) * P], ident[:])
            ht = htp.tile([P, NCH, P], MD, tag="ht")
            nc.scalar.activation(ht[:], trt[:], AF.Copy)
            hT[(0, t)] = ht

            # gate for this tile (batch-major logits)
            lg = zp.tile([P, D], F32, tag="z")
            for c in range(NCH):
                nc.tensor.matmul(
                    lg[:, :K],
                    ht[:, c, :],
                    gw[:, c * K:(c + 1) * K],
                    start=(c == 0),
                    stop=(c == NCH - 1),
                )
            ex = smp.tile([P, K], F32, tag="ex")
            if has_gate_b:
                nc.vector.scalar_tensor_tensor(
                    ex[:], lg[:, :K], 1.0, gb_bc[:], op0=ALU.mult, op1=ALU.add
                )
                nc.scalar.activation(ex[:], ex[:], AF.Exp)
            else:
                nc.scalar.activation(ex[:], lg[:, :K], AF.Exp)
            ssum = smp.tile([P, 1], F32, tag="ssum")
            nc.vector.reduce_sum(ssum[:], ex[:], axis=mybir.AxisListType.X)
            rec = smp.tile([P, 1], F32, tag="rec")
            nc.vector.reciprocal(rec[:], ssum[:])
            nc.vector.tensor_scalar_mul(alpha[:, t * K:(t + 1) * K], ex[:], rec[:])

            if any_bias:
                # alphaT[:, t*128:(t+1)*128] = alpha_tile.T (8 x 128)
                at_ps = trp.tile([P, NCH, P], F32, tag="tr")
                nc.tensor.transpose(
                    at_ps[:K, 0, :], alpha[:, t * K:(t + 1) * K], ident[:]
                )
                nc.scalar.activation(
                    alphaT[:, t * P:(t + 1) * P], at_ps[:K, 0, :], AF.Copy
                )

        # ---- 4 MoIE blocks (x repeat for timing builds) ----
        for gli in range(4 * repeat):
            li = gli % 4
            last = gli == 4 * repeat - 1
            # stream this layer's weights (reused across all 16 batch tiles)
            wt = {}
            for k in range(K):
                for c in range(NCH):
                    w_t = wpool.tile([P, D], MD, tag="w", name=f"w_{gli}_{k}_{c}")
                    nc.sync.dma_start(w_t[:], ws[li][k, c * P:(c + 1) * P, :])
                    wt[(k, c)] = w_t

            for t in range(NT):
                bias_sb = None
                if has_bias[li]:
                    b_ps = trp.tile([P, D], F32, tag="tr")
                    nc.tensor.matmul(
                        b_ps[:],
                        alphaT[:, t * P:(t + 1) * P],
                        bl_sb[li][:],
                    )
                    bias_sb = smp.tile([P, D], F32, tag="bias_sb")
                    nc.scalar.activation(bias_sb[:], b_ps[:], AF.Copy)

                ht_in = hT[(gli, t)] if mode == "full" or mode == "no_stt" else hT[(0, t)]
                zs = []
                for k in range(K):
                    z = zp.tile([P, D], F32, tag="z", name=f"z_{gli}_{t}_{k}")
                    for c in range(NCH):
                        lhs = ht_in[:, 0, :] if mode == "pe_same" else ht_in[:, c, :]
                        nc.tensor.matmul(
                            z[:],
                            lhs,
                            wt[(k, c)][:],
                            start=(c == 0),
                            stop=(c == NCH - 1),
                        )
                    zs.append(z)

                if mode in ("pe_only", "pe_same"):
                    continue

                # weighted sum over experts: VectorE STT chain, optionally
                # with the tail experts scaled on ScalarE + summed on GpSimd
                acc = accp.tile([P, D], F32, tag="acc")
                n_dve = K - gp_offload
                if mode == "no_stt":
                    nc.scalar.activation(acc[:], zs[K - 1][:], AF.Copy)
                else:
                    gsum = None
                    for j, k in enumerate(range(n_dve, K)):
                        a_ap = alpha[:, t * K + k:t * K + k + 1]
                        sk = smp.tile([P, D], F32, tag="sk", bufs=3)
                        nc.scalar.activation(sk[:], zs[k][:], AF.Copy, scale=a_ap)
                        if j == 0:
                            gsum = sk
                        else:
                            gs2 = smp.tile([P, D], F32, tag="gs", bufs=2)
                            nc.gpsimd.tensor_add(gs2[:], gsum[:], sk[:])
                            gsum = gs2
                    for k in range(n_dve):
                        a_ap = alpha[:, t * K + k:t * K + k + 1]
                        if k == 0:
                            if bias_sb is not None:
                                nc.vector.scalar_tensor_tensor(
                                    acc[:], zs[0][:], a_ap, bias_sb[:],
                                    op0=ALU.mult, op1=ALU.add,
                                )
                            else:
                                nc.vector.tensor_scalar_mul(acc[:], zs[0][:], a_ap)
                        else:
                            nc.vector.scalar_tensor_tensor(
                                acc[:], zs[k][:], a_ap, acc[:],
                                op0=ALU.mult, op1=ALU.add,
                            )
                    if gsum is not None:
                        nc.vector.tensor_add(acc[:], acc[:], gsum[:])

                if not last:
                    # relu commutes with the transpose: fuse it into the
                    # PSUM->SBUF eviction copy instead of a separate pass
                    if mode in ("full", "no_stt"):
                        trt = trp.tile([P, NCH, P], F32, tag="tr")
                        for c in range(NCH):
                            nc.tensor.transpose(
                                trt[:, c, :], acc[:, c * P:(c + 1) * P], ident[:]
                            )
                        ht_n = htp.tile([P, NCH, P], MD, tag="ht")
                        nc.scalar.activation(
                            ht_n[:], trt[:], AF.Relu if li < 3 else AF.Copy
                        )
                        hT[(gli + 1, t)] = ht_n
                    elif li < 3:
                        nc.scalar.activation(acc[:], acc[:], AF.Relu)
                else:
                    nc.sync.dma_start(out[t * P:(t + 1) * P, :], acc[:])

    nc.compile()
    return nc


_CACHE = {}


def _get_nc(key):
    if key not in _CACHE:
        _CACHE[key] = _build(key[0], key[1])
    return _CACHE[key]


def kernel(**inputs):
    x = np.ascontiguousarray(np.asarray(inputs["x"], dtype=np.float32))
    gate_w = np.ascontiguousarray(np.asarray(inputs["gate_w"], dtype=np.float32))
    gate_b = np.ascontiguousarray(np.asarray(inputs["gate_b"], dtype=np.float32))
    wlist = [np.ascontiguousarray(np.asarray(inputs[n], dtype=np.float32)) for n in W_NAMES]
    blist = [np.ascontiguousarray(np.asarray(inputs[n], dtype=np.float32)) for n in B_NAMES]

    has_gate_b = bool(np.any(gate_b))
    has_bias = tuple(bool(np.any(b)) for b in blist)
    nc = _get_nc((has_gate_b, has_bias))

    shared = {"gate_w": gate_w, "gate_b": gate_b}
    for n, w in zip(W_NAMES, wlist):
        shared[n] = w
    for n, b in zip(B_NAMES, blist):
        shared[n] = b

    core_ids = list(range(NCORES))
    in_maps = [dict(shared, x=x[i * R:(i + 1) * R]) for i in core_ids]
    res = run_bass_kernel_spmd(nc, in_maps, core_ids)
    return np.concatenate([res.results[i]["out"] for i in core_ids], axis=0)


if __name__ == "__main__":
    rng = np.random.default_rng(0)
    ins = {
        "x": rng.standard_normal((B, D), dtype=np.float32),
        "gate_w": rng.standard_normal((D, K), dtype=np.float32) * 0.02,
        "gate_b": np.zeros((K,), np.float32),
    }
    for n in W_NAMES:
        ins[n] = rng.standard_normal((K, D, D), dtype=np.float32) * 0.02
    for n in B_NAMES:
        ins[n] = np.zeros((K, D), np.float32)
    y = kernel(**ins)
    print("out", y.shape, y.dtype, float(np.abs(y).max()))
